# revision 23
# baseline (speedup 1.0000x reference)
"""Multi-head attention kernel for Trainium2, 8 NeuronCores.

Problem (NHEAD=8, T=S=1024, B=8, A=512, hd=64):
  q = queries.reshape(T, B*NH, hd); k = keys.reshape(S, B*NH, hd)
  w = softmax(mask(q @ k^T / sqrt(hd)))      per n = b*NH + h, mask = attn_mask[n % NH]
  out = (w @ k).reshape(T, B, A)             (keys double as values)

Sharding: head-parallel. Core c owns head h=c for all 8 batches.

v4 design — decoupled dual softmax pipelines:
  - A-path (13 of 16 (st,th) blocks x 4 batch-pairs = 52 tiles): ACT exp
    (exact spline, scale ln2/128) -> bf16, then a DVE 2x-mode merged mask
    multiply.  Runs as a convoy of 7 stages x 4 pairs over a 2-slot PSUM
    ring (4 banks); sc slots are freed by the exps themselves, so the
    convoy is back-to-back exps (~53us) regardless of DVE lag.
  - B-path (3 blocks = 12 tiles): Schraudolph bit-trick exp entirely on
    DVE, decoupled on its OWN 1-bank PSUM ring so it never perturbs the
    A-convoy: per b01-half, one K=64 matmul then ONE tensor_tensor
    i16 = rne(sc + m16) where qt is pre-scaled by ALPHA=16*log2e
    (sc = ALPHA*w) and m16 = 16250 (keep) / 10106 (masked).  The i16 bit
    pattern IS bf16 2^((i-16256)/128) ~ exp(w/8) (masked -> ~2^-54); mm2
    reads the i16 tile bitcast as bf16.  (Rounding mode and bitcast
    matmul verified exact on HW.)  B work has no stage deadline — only
    pair-end normalize — so DVE FIFO slack absorbs it.
  - mm2 accumulator: 3 PSUM banks per pair, 16 (t-block, b01) x 65-col
    slots packed 6/6/4 (col = (q%6)*65, bank q//6, q = tb*2+b01, col 64
    of each slot accumulates the softmax denominator via a ones column
    in kn).  Normalize: ONE reciprocal (padded uniform [3,6] grid — two
    garbage slots, ignored) + ONE broadcast multiply per pair, junk
    rides to the host which strips it.  PSUM: 4 (A ring) + 1 (B) + 3 = 8.
  - Pair-final stages run unmerged masks and their mm2 at skew-1 so each
    pair's normalize releases the accumulator banks before the next
    pair's first (start=True) mm2 needs them.
  Accuracy: rel_err ~4e-3 vs the 2e-2 gate (see sim_schraudolph.py).
"""

import numpy as np
import ml_dtypes

import concourse.bass as bass
import concourse.mybir as mybir
import concourse.tile as tile
from concourse.bass_utils import run_bass_kernel_spmd

BF16 = ml_dtypes.bfloat16

T = 1024
S = 1024
B = 8
NH = 8
HD = 64
N_CORES = 8
ALPHA = float(16.0 * np.log2(np.e))   # qt pre-scale: sc = ALPHA * w
S_EXP = float(np.log(2.0) / 128.0)    # ACT exp scale: exp(S_EXP * sc) = exp(w/8)
M16_KEEP = 16250                      # 16256 + center(-6); round-to-nearest
M16_MASK = 10106                      # bitcast -> ~2^-54: masked entries vanish

# (st, th) blocks on the DVE Schraudolph path.
B_LIST = [(2, 0), (4, 1), (6, 0)]
B_SET = set(B_LIST)
# A-path stages (7 per batch-pair): first is a single-tile stage (odd
# count), the rest two-tile merged stages.
STAGES = [
    [(0, 0)],
    [(0, 1), (1, 0)],
    [(1, 1), (2, 1)],
    [(3, 0), (3, 1)],
    [(4, 0), (5, 0)],
    [(5, 1), (6, 1)],
    [(7, 0), (7, 1)],
]

# Empirical per-instruction sem-wait limit for this walrus build: even a
# Matmult with 2 waits fails codegen ("Too many sync wait commands"), so
# every instruction keeps at most one inline wait.
_WAIT_LIMITS = {}


def _split_excess_waits(nc, default_max=1):
    """Hoist excess sem waits off instructions onto standalone
    EventSemaphore waits placed just before them on the same engine queue —
    semantically identical, since each engine executes its queue in order."""
    n = 0
    for f in nc.m.functions:
        for bb in f.blocks:
            insts = bb.instructions
            out = []
            changed = False
            for ins in insts:
                si = ins.sync_info
                waits = list(si.on_wait) if si is not None and si.on_wait else []
                max_waits = _WAIT_LIMITS.get(type(ins).__name__, default_max)
                if (
                    len(waits) > max_waits
                    and type(ins).__name__ != "InstEventSemaphore"
                ):
                    changed = True
                    for w in waits[:-max_waits]:
                        n += 1
                        we = mybir.InstEventSemaphore(
                            name=f"WSPLIT-{n}", ins=[], outs=[]
                        )
                        we.engine = ins.engine
                        we.sync_info = mybir.SyncInfo(on_wait=[w], on_update=[])
                        nc.register_instruction(we)
                        out.append(we)
                    ins.sync_info = mybir.SyncInfo(
                        on_wait=waits[-max_waits:],
                        on_update=list(si.on_update) if si.on_update else [],
                    )
                out.append(ins)
            if changed:
                bb.instructions = out


def build_nc():
    fp32 = mybir.dt.float32
    bf16 = mybir.dt.bfloat16
    i16 = mybir.dt.int16

    nc = bass.Bass(target_bir_lowering=False)
    # Per-core inputs (host pre-sliced/cast/transposed; SPMD: same program,
    # per-core data). qt/kt rows are (b, h) pairs: rows 128p..128p+127 hold
    # batches 2p (partitions 0-63) and 2p+1 (partitions 64-127).
    qt_in = nc.dram_tensor("qt", [B * HD, T], bf16, kind="ExternalInput")
    kt_in = nc.dram_tensor("kt", [B * HD, S], bf16, kind="ExternalInput")
    knat = nc.dram_tensor("knat", [S, B * HD], bf16, kind="ExternalInput")
    maskt = nc.dram_tensor("maskt", [S, T], bf16, kind="ExternalInput")
    m16t = nc.dram_tensor("m16t", [S, T], i16, kind="ExternalInput")
    # out[p, pair, q-slot layout]: q = tb*2 + b01 (tb = th*4+k),
    # bank j = q//6, col = j*390 + (q%6)*65 + c; c=64 junk denominator,
    # bank-2 slots 4-5 garbage.  t = tb*128 + p, b = 2*pair + b01, h = c.
    out = nc.dram_tensor("out", [128, 4, 1170], bf16, kind="ExternalOutput")

    knat3 = knat.rearrange("(st p) (b h) -> st p b h", p=128, b=B)

    with tile.TileContext(nc) as tc:
        with (
            tc.tile_pool(name="consts", bufs=1) as consts,
            tc.tile_pool(name="ptp", bufs=6) as ptp,
            tc.tile_pool(name="pte", bufs=6) as pte,
            tc.tile_pool(name="ptbp", bufs=4) as ptbp,
            tc.tile_pool(name="rcp", bufs=3) as rcp,
            tc.tile_pool(name="scp", bufs=2, space="PSUM") as scp,
            tc.tile_pool(name="scbp", bufs=1, space="PSUM") as scbp,
            tc.tile_pool(name="opp", bufs=1, space="PSUM") as opp,
        ):
            kt = [consts.tile([128, S], bf16, tag=f"kt{p}", name=f"kt{p}") for p in range(4)]
            qt = [consts.tile([128, T], bf16, tag=f"qt{p}", name=f"qt{p}") for p in range(4)]
            # A-stage mask tiles (one [128, 512*len(stage)] bf16 tile per
            # stage; blocks DMA'd into halves), i16 tiles per B block.
            mst = [
                consts.tile(
                    [128, 512 * len(stage)], bf16, tag=f"mst{si}", name=f"mst{si}"
                )
                for si, stage in enumerate(STAGES)
            ]
            mbt = {
                blk: consts.tile(
                    [128, 512], i16, tag=f"m16_{blk[0]}_{blk[1]}",
                    name=f"m16_{blk[0]}_{blk[1]}",
                )
                for blk in B_LIST
            }
            kn = [
                consts.tile([128, B, HD + 1], bf16, tag=f"kn{s}", name=f"kn{s}")
                for s in range(8)
            ]
            outt = consts.tile([128, 4, 1170], bf16, tag="outt", name="outt")

            # warm the ACT exp table first on the Activation queue, then
            # the two loads mm1(stage0) actually needs, then bulk kt0/kn0 —
            # all issuing in parallel with the SP-queue stream.
            wsrc = consts.tile([128, 1], mybir.dt.float32, tag="wsrc", name="wsrc")
            wdst = consts.tile([128, 1], bf16, tag="wdst", name="wdst")
            nc.vector.memset(wsrc[:], 0.0)
            # the two loads mm1(stage0,tile0) needs go FIRST on the scalar
            # queue — ahead of the warm exp, whose implicit ~1.3us
            # ACT_TABLE_LOAD would otherwise delay their DIRECT2D issue.
            nc.scalar.dma_start(out=kt[0][:, 0:128], in_=kt_in[0:128, 0:128])
            nc.scalar.dma_start(out=qt[0][:, 0:512], in_=qt_in[0:128, 0:512])
            nc.scalar.activation(
                wdst[:], wsrc[:], mybir.ActivationFunctionType.Exp
            )
            nc.scalar.dma_start(out=kt[0][:, 128:512], in_=kt_in[0:128, 128:512])
            nc.scalar.dma_start(out=kt[0][:, 512:1024], in_=kt_in[0:128, 512:1024])
            nc.vector.memset(kn[0][:, :, HD], 1.0)
            nc.scalar.dma_start(out=kn[0][:, :, 0:HD], in_=knat3[0])

            # SP HWDGE queue order IS the DMA service order; primer DMA
            # rings the doorbell early.
            dummy = consts.tile([1, 64], bf16, tag="dummy", name="dummy")
            nc.sync.dma_start(out=dummy[:], in_=qt_in[0:1, 0:64])
            nc.sync.dma_start(out=qt[0][:, 512:1024], in_=qt_in[0:128, 512:1024])

            def load_stage_mask(si):
                for i, (st, th) in enumerate(STAGES[si]):
                    nc.sync.dma_start(
                        out=mst[si][:, i * 512 : (i + 1) * 512],
                        in_=maskt[
                            st * 128 : (st + 1) * 128, th * 512 : (th + 1) * 512
                        ],
                    )

            # stage masks + B masks + kn + kt/qt prefetch, ordered by first
            # use (B(2,0) is consumed from iteration (0,1) on).
            def load_kn(st):
                nc.vector.memset(kn[st][:, :, HD], 1.0)
                nc.sync.dma_start(out=kn[st][:, :, 0:HD], in_=knat3[st])

            load_stage_mask(0)
            load_stage_mask(1)
            nc.sync.dma_start(
                out=mbt[(2, 0)][:], in_=m16t[2 * 128 : 3 * 128, 0:512]
            )
            load_kn(2)  # first mm2 consumer is B(2,0) at iteration 2
            load_stage_mask(2)
            load_kn(1)
            load_stage_mask(3)
            nc.sync.dma_start(
                out=mbt[(4, 1)][:], in_=m16t[4 * 128 : 5 * 128, 512:1024]
            )
            load_kn(3)
            load_stage_mask(4)
            nc.sync.dma_start(out=kt[1][:], in_=kt_in[128:256, :])
            nc.sync.dma_start(out=qt[1][:], in_=qt_in[128:256, :])
            load_kn(4)
            nc.sync.dma_start(
                out=mbt[(6, 0)][:], in_=m16t[6 * 128 : 7 * 128, 0:512]
            )
            load_stage_mask(5)
            load_kn(5)
            load_stage_mask(6)
            nc.sync.dma_start(out=kt[2][:], in_=kt_in[256:384, :])
            nc.sync.dma_start(out=qt[2][:], in_=qt_in[256:384, :])
            load_kn(6)
            load_kn(7)
            nc.sync.dma_start(out=kt[3][:], in_=kt_in[384:512, :])
            nc.sync.dma_start(out=qt[3][:], in_=qt_in[384:512, :])

            # ---------------- emission helpers ----------------
            ops = [None] * 4      # per-pair 3-bank accumulator
            started = [set() for _ in range(4)]  # banks already start=True'd

            def emit_mm1(pair, st, th):
                sc = scp.tile(
                    [128, 2, 512], fp32, tag="sc", name=f"sc_{pair}_{st}_{th}"
                )
                for b01 in range(2):
                    nc.tensor.matmul(
                        sc[:, b01, :],
                        kt[pair][b01 * 64 : (b01 + 1) * 64, st * 128 : (st + 1) * 128],
                        qt[pair][b01 * 64 : (b01 + 1) * 64, th * 512 : (th + 1) * 512],
                        start=True,
                        stop=True,
                        tile_position=(b01 * 64, 0),
                    )
                return sc

            def emit_front(pair, si):
                stage = STAGES[si]
                if len(stage) == 1 or si == 6:
                    # unmerged: single-tile stage, or the pair-final stage
                    # (whose pt halves feed the skew-1 mm2 sooner).
                    res = []
                    for st, th in stage:
                        sc = emit_mm1(pair, st, th)
                        pe = pte.tile(
                            [128, 2, 512], bf16, tag="pea",
                            name=f"pe_{pair}_{st}_{th}",
                        )
                        nc.scalar.activation(
                            pe[:], sc[:], mybir.ActivationFunctionType.Exp,
                            scale=S_EXP,
                        )
                        pt = ptp.tile(
                            [128, 2, 512], i16, tag="pta",
                            name=f"pt_{pair}_{st}_{th}",
                        )
                        i = stage.index((st, th))
                        mbc = (
                            mst[si][:, i * 512 : (i + 1) * 512]
                            .rearrange("p (o x) -> p o x", o=1)
                            .to_broadcast([128, 2, 512])
                        )
                        nc.vector.tensor_tensor(
                            out=pt[:].bitcast(bf16), in0=pe[:], in1=mbc,
                            op=mybir.AluOpType.mult,
                        )
                        res.append((st, th, pt, None))
                    return res
                # merged two-tile stage
                pe = pte.tile(
                    [128, 2, 2, 512], bf16, tag="pem", name=f"pe_{pair}_{si}"
                )
                for i, (st, th) in enumerate(stage):
                    sc = emit_mm1(pair, st, th)
                    nc.scalar.activation(
                        pe[:, i], sc[:], mybir.ActivationFunctionType.Exp,
                        scale=S_EXP,
                    )
                pt = ptp.tile(
                    [128, 2, 2, 512], i16, tag="ptm", name=f"pt_{pair}_{si}"
                )
                mbc = (
                    mst[si][:]
                    .rearrange("p (t x) -> p t x", t=2)
                    .rearrange("p t (o x) -> p t o x", o=1)
                    .to_broadcast([128, 2, 2, 512])
                )
                nc.vector.tensor_tensor(
                    out=pt[:].bitcast(bf16), in0=pe[:], in1=mbc,
                    op=mybir.AluOpType.mult,
                )
                return [
                    (st, th, pt, i) for i, (st, th) in enumerate(stage)
                ]

            def alloc_ops(pair):
                if ops[pair] is None:
                    ops[pair] = opp.tile(
                        [128, 3, 512], fp32, tag="op", name=f"op_{pair}"
                    )

            def mm2_block(pair, st, th, b01, lhsT):
                # one (t-block, b01) 65-col slot: q = tb*2+b01
                alloc_ops(pair)
                for k in range(4):
                    q = (th * 4 + k) * 2 + b01
                    j, slot = q // 6, q % 6
                    st_flag = j not in started[pair]
                    started[pair].add(j)
                    nc.tensor.matmul(
                        ops[pair][:, j, slot * 65 : (slot + 1) * 65],
                        lhsT[:, k * 128 : (k + 1) * 128].bitcast(bf16),
                        kn[st][:, pair * 2 + b01, :],
                        start=st_flag,
                        stop=(st == 7),
                        skip_group_check=True,
                    )

            def emit_mm2_front(pair, fr):
                for st, th, pt, sel in fr:
                    for b01 in range(2):
                        v = pt[:, sel, b01] if sel is not None else pt[:, b01]
                        mm2_block(pair, st, th, b01, v)

            def emit_b_half(pair, bi, b01):
                st, th = B_LIST[bi]
                scb = scbp.tile([128, 512], fp32, tag="scb",
                                name=f"scb_{pair}_{bi}_{b01}")
                nc.tensor.matmul(
                    scb[:],
                    kt[pair][b01 * 64 : (b01 + 1) * 64, st * 128 : (st + 1) * 128],
                    qt[pair][b01 * 64 : (b01 + 1) * 64, th * 512 : (th + 1) * 512],
                    start=True,
                    stop=True,
                    tile_position=(b01 * 64, 0),
                )
                ptb = ptbp.tile([128, 512], i16, tag="ptb",
                                name=f"ptb_{pair}_{bi}_{b01}")
                nc.vector.tensor_tensor(
                    out=ptb[:], in0=scb[:], in1=mbt[(st, th)][:],
                    op=mybir.AluOpType.add,
                )
                return (pair, st, th, ptb, b01)

            def emit_b_mm2(work):
                pair, st, th, ptb, b01 = work
                mm2_block(pair, st, th, b01, ptb[:])

            def emit_recip(pair):
                den = (
                    ops[pair][:, :, 0:390]
                    .rearrange("p j (q c) -> p j q c", c=65)[:, :, :, HD]
                )
                rc = rcp.tile([128, 3, 6, 1], fp32, tag="rc", name=f"rc_{pair}")
                nc.vector.reciprocal(rc[:, :, :, 0], den)
                return rc

            def emit_norm(pair, rc, j0, j1, queue):
                opv = ops[pair][:, j0:j1, 0:390].rearrange(
                    "p j (q c) -> p j q c", c=65
                )
                ov = outt[:, pair, j0 * 390 : j1 * 390].rearrange(
                    "p (j q c) -> p j q c", q=6, c=65
                )
                nc.vector.tensor_tensor(
                    out=ov, in0=opv,
                    in1=rc[:, j0:j1].to_broadcast([128, j1 - j0, 6, 65]),
                    op=mybir.AluOpType.mult,
                )
                queue.dma_start(
                    out=out[:, pair, j0 * 390 : j1 * 390],
                    in_=outt[:, pair, j0 * 390 : j1 * 390],
                )

            # ---------------- main loop ----------------
            from collections import deque

            fronts = {}       # G -> front result
            bqueue = deque()  # B mm2 work pending one iteration
            pending = deque() # normalize pieces

            def iteration(G):
                pair, si = divmod(G, 7)
                # 1. front stage
                fronts[G] = emit_front(pair, si)
                # 2. one pending normalize piece
                if pending:
                    pending.popleft()()
                # 3. B half (si 1..6 -> halves 0..5)
                if si >= 1:
                    bqueue.append(emit_b_half(pair, (si - 1) // 2, (si - 1) % 2))
                # 4. B mm2 from the previous iteration's half
                while len(bqueue) > (1 if si >= 1 else 0):
                    emit_b_mm2(bqueue.popleft())
                # 5. front mm2: skew-2, plus pair-final stages at skew-1
                for Gm in (G - 2, G - 1):
                    if Gm < 0 or Gm not in fronts:
                        continue
                    pm, sm = divmod(Gm, 7)
                    if Gm == G - 2 or sm == 6:
                        emit_mm2_front(pm, fronts.pop(Gm))
                        if sm == 6:
                            rc = emit_recip(pm)
                            pending.append(
                                lambda p=pm, r=rc: emit_norm(p, r, 0, 2, nc.sync)
                            )
                            pending.append(
                                lambda p=pm, r=rc: emit_norm(p, r, 2, 3, nc.sync)
                            )

            for G in range(28):
                iteration(G)

            # epilogue: last B half's mm2, final stages' mm2, pair-3 norm
            while bqueue:
                emit_b_mm2(bqueue.popleft())
            for Gm in (26, 27):
                if Gm in fronts:
                    emit_mm2_front(3, fronts.pop(Gm))
            rc = emit_recip(3)
            emit_norm(3, rc, 0, 1, nc.scalar)
            emit_norm(3, rc, 1, 2, nc.sync)
            emit_norm(3, rc, 2, 3, nc.scalar)

    _split_excess_waits(nc)
    return nc


_NC_CACHE = None


def _get_nc():
    global _NC_CACHE
    if _NC_CACHE is None:
        _NC_CACHE = build_nc()
    return _NC_CACHE


def kernel(queries: np.ndarray, keys: np.ndarray, attn_mask: np.ndarray) -> np.ndarray:
    assert queries.shape == (T, B, NH * HD)
    assert keys.shape == (S, B, NH * HD)
    assert attn_mask.shape == (B, T, S)

    q_bf = (np.asarray(queries, np.float32) * np.float32(ALPHA)).astype(BF16)
    k_bf = np.asarray(keys, np.float32).astype(BF16)
    m_bf = np.asarray(attn_mask).astype(BF16)  # bool -> 0.0/1.0
    m16 = np.where(np.asarray(attn_mask), M16_KEEP, M16_MASK).astype(np.int16)

    in_maps = []
    for c in range(N_CORES):
        qs = q_bf[:, :, c * HD : (c + 1) * HD].reshape(T, B * HD)  # [T,(b,h)]
        ks = k_bf[:, :, c * HD : (c + 1) * HD].reshape(S, B * HD)
        in_maps.append(
            {
                "qt": np.ascontiguousarray(qs.T),
                "kt": np.ascontiguousarray(ks.T),
                "knat": np.ascontiguousarray(ks),
                "maskt": np.ascontiguousarray(m_bf[c].T),
                "m16t": np.ascontiguousarray(m16[c].T),
            }
        )

    nc = _get_nc()
    res = run_bass_kernel_spmd(nc, in_maps, core_ids=list(range(N_CORES)))
    kernel.last_results = res

    outp = np.empty((T, B, NH * HD), np.float32)
    for c in range(N_CORES):
        arr = res.results[c]["out"].astype(np.float32)  # [128, 4, 1170]
        v = arr.reshape(128, 4, 18, 65)[..., :64]  # [p, pair, j*6+slot, h]
        # q = tb*2 + b01 at flat slot j*6+(q%6) == q for q<16
        v = v[:, :, :16].reshape(128, 4, 8, 2, 64)  # [p, pair, tb, b01, h]
        v = v.transpose(2, 0, 1, 3, 4).reshape(T, B, HD)
        outp[:, :, c * HD : (c + 1) * HD] = v
    return outp
